# revision 1
# baseline (speedup 1.0000x reference)
"""AdaptiveClusteringAttention TRN2 kernel (v4).

Data-parallel over batch: b=8 rows -> 8 NeuronCores, one row per core,
weights replicated. No collectives.

Per-core math (n=4096 tokens, d=1024, C=256 clusters, H=16 heads, dh=64):
  xc[c,:]  = sum_{t: cluster[t]=c} x[t,:]            (onehot matmul, raw sums)
  cnt[c]   = |{t: cluster[t]=c}|
  kc       = (xc / max(cnt,.5)) @ w_k                (mean k per cluster)
  vc'      = xc @ w_v                                (= cnt * v_center!)
  qh       = x @ w_q
  s[t,c]   = qh_h[t] . kc_h[c] / 8
  out      = (exp(s) @ vc'_h) / (exp(s) . cnt)       (count-weighted softmax)
  y        = out @ w_proj + b_proj

The count-weighting is folded into vc' (no log-count softmax bias), so exp
needs no per-partition bias and both cluster halves batch into one ACT call.
The per-token denominator comes free from a cnt column appended to vc'.

Precision: fp8 is confined to the score path (x^T, w_q, qh, kc) where
softmax sensitivity suppresses its ~2.8%/tensor quantization noise; the
value path stays bf16 (matmul operand quantization noise passes through to
the output at full strength). qh runs as fp8 DoubleRow during the phase-A
DMA stream; x^T comes from a bf16 DRAM round trip + XBAR DMA-transpose.

Phase C is software-pipelined: chunk ch-1's output projection and 1/sumexp
broadcast are interleaved into chunk ch's attention-head loop so the PE
stays dense (HAM stays at K=8/8) while ACT computes the exps.
"""

import os
import sys

import numpy as np

for _p in ("/opt/trn_rl_repo", os.path.expanduser("~/.axon_site/_ro/trn_rl_repo")):
    if os.path.isdir(_p) and _p not in sys.path:
        sys.path.append(_p)

import concourse.bass as bass  # noqa: E402
import concourse.mybir as mybir  # noqa: E402
import concourse.tile as tile  # noqa: E402
from concourse import bacc  # noqa: E402
from concourse.masks import make_identity  # noqa: E402

FP32 = mybir.dt.float32
BF16 = mybir.dt.bfloat16
F8 = mybir.dt.float8e4
I32 = mybir.dt.int32
DR = mybir.MatmulPerfMode.DoubleRow

N, D, C, H, DH, P = 4096, 1024, 256, 16, 64, 128
NJ = N // P          # 32 token row-tiles
NK = D // P          # 8 contraction chunks
TCH = 512            # token chunk for the attention phase
NCH = N // TCH       # 8 chunks
NMT = TCH // P       # 4 token subtiles per chunk

TRACE = False
LAST_RESULTS = None


def build_nc():
    nc = bacc.Bacc("TRN2", target_bir_lowering=False, debug=False)

    x_d = nc.dram_tensor("x", [N, D], FP32, kind="ExternalInput").ap()
    cl_d = nc.dram_tensor("cluster", [N], I32, kind="ExternalInput").ap()
    wq_d = nc.dram_tensor("w_q", [D, D], FP32, kind="ExternalInput").ap()
    wk_d = nc.dram_tensor("w_k", [D, D], FP32, kind="ExternalInput").ap()
    wv_d = nc.dram_tensor("w_v", [D, D], FP32, kind="ExternalInput").ap()
    wp_d = nc.dram_tensor("w_proj", [D, D], FP32, kind="ExternalInput").ap()
    bp_d = nc.dram_tensor("b_proj", [1, D], FP32, kind="ExternalInput").ap()
    out_d = nc.dram_tensor("out", [N, D], FP32, kind="ExternalOutput").ap()

    with tile.TileContext(nc) as tc:
        with tc.tile_pool(name="wts", bufs=1) as wts:
            # ---- constants ----
            iota_i = wts.tile([P, C], I32, tag="iota_i")
            nc.gpsimd.iota(iota_i[:], pattern=[[1, C]], base=0, channel_multiplier=0)
            iota_b = wts.tile([P, C], BF16, tag="iota_b")
            nc.vector.tensor_copy(iota_b[:], iota_i[:])
            ident32 = wts.tile([32, 32], BF16, tag="ident32")
            make_identity(nc, ident32[:])
            ident128 = wts.tile([P, P], BF16, tag="ident128")
            make_identity(nc, ident128[:])
            one11 = wts.tile([1, 1], BF16, tag="one11")
            nc.vector.memset(one11[:], 1.0)
            ones_col = wts.tile([P, 1], BF16, tag="ones_col")
            nc.vector.memset(ones_col[:], 1.0)
            ones_row = wts.tile([1, 64], BF16, tag="ones_row")
            nc.vector.memset(ones_row[:], 1.0)
            ones16 = wts.tile([P, 16], BF16, tag="ones16")
            nc.vector.memset(ones16[:], 1.0)
            ones16v = ones16.rearrange("p (h e) -> p h e", e=1)

            bp_sb = wts.tile([1, D], BF16, tag="bp_sb")
            nc.gpsimd.dma_start(out=bp_sb[:], in_=bp_d)
            b_bc = wts.tile([P, D], BF16, tag="b_bc")
            nc.gpsimd.partition_broadcast(b_bc[:], bp_sb[:])

            cl_i = wts.tile([NJ, P], I32, tag="cl_i")
            nc.sync.dma_start(out=cl_i[:], in_=cl_d.rearrange("(a b) -> a b", b=P))
            cl_b = wts.tile([NJ, P], BF16, tag="cl_b")
            nc.vector.tensor_copy(cl_b[:], cl_i[:])
            clusT = wts.tile([P, NJ], FP32, tag="clusT")
            with tc.tile_pool(name="psct", bufs=1, space="PSUM") as psct:
                ct_ps = psct.tile([P, NJ], BF16, tag="ct")
                nc.tensor.transpose(ct_ps[:], cl_b[:], ident32[:])
                nc.vector.tensor_copy(clusT[:], ct_ps[:])

            # ---- persistent data tiles ----
            wq8 = wts.tile([P, NK * D], F8, tag="wq8")
            wq8v = wq8.rearrange("p (k n) -> p k n", n=D)

            qh8 = [wts.tile([P, TCH], F8, tag=f"qh{i}", name=f"qh{i}")
                   for i in range(NCH * NK)]
            kc8 = [wts.tile([P, C], F8, tag=f"kc{m}", name=f"kc{m}")
                   for m in range(NK)]
            vca = [wts.tile([P, H * 65], BF16, tag=f"vca{i}", name=f"vca{i}")
                   for i in range(2)]
            xcm = [wts.tile([P, C], BF16, tag=f"xcm{m}", name=f"xcm{m}")
                   for m in range(NK)]
            xcr = [wts.tile([P, C], BF16, tag=f"xcr{m}", name=f"xcr{m}")
                   for m in range(NK)]

            cnt_sb = wts.tile([1, C], FP32, tag="cnt_sb")
            cnt_bf = wts.tile([1, C], BF16, tag="cnt_bf")
            cm_row = wts.tile([1, C], FP32, tag="cm_row")
            inv_row = wts.tile([1, C], FP32, tag="inv_row")
            inv_bc = wts.tile([P, C], FP32, tag="inv_bc")
            cnt_col = wts.tile([P, 2], FP32, tag="cnt_col")

            wk_sb = [wts.tile([P, D], BF16, tag=f"wk{k}", name=f"wk{k}")
                     for k in range(NK)]
            wv_sb = [wts.tile([P, D], BF16, tag=f"wv{k}", name=f"wv{k}")
                     for k in range(NK)]

            # ---- phase A: stream x, counts, x^T round trip, qh, xc ----
            with (
                tc.tile_pool(name="xin", bufs=1) as xin,
                tc.tile_pool(name="ohp", bufs=1) as ohp,
                tc.tile_pool(name="xtp8", bufs=2) as xtp8,
                tc.tile_pool(name="wst", bufs=2) as wst,
                tc.tile_pool(name="psA", bufs=1, space="PSUM") as psA,
                tc.tile_pool(name="psAm", bufs=2, space="PSUM") as psAm,
                tc.tile_pool(name="psT", bufs=1, space="PSUM") as psT,
                tc.tile_pool(name="psTT", bufs=2, space="PSUM") as psTT,
                tc.tile_pool(name="psq", bufs=2, space="PSUM") as psq,
            ):
                # w_q first (qh needs it immediately): bf16 cast-DMA -> fp8
                for k in range(NK):
                    st = wst.tile([P, D], BF16, tag="wst")
                    nc.gpsimd.dma_start(out=st[:], in_=wq_d[k * P:(k + 1) * P, :])
                    nc.scalar.copy(wq8v[:, k:k + 1, :], st[:])

                pcnt = psA.tile([1, C], FP32, tag="cnt")
                xall, ohall = [], []
                for ch in range(NCH):
                    for jj in range(NMT):
                        j = ch * NMT + jj
                        xj = xin.tile([P, D], BF16, tag=f"xj{j}", name=f"xj{j}")
                        nc.gpsimd.dma_start(out=xj[:], in_=x_d[j * P:(j + 1) * P, :])
                        oh = ohp.tile([P, C], BF16, tag=f"oh{j}", name=f"oh{j}")
                        nc.vector.tensor_scalar(
                            oh[:], iota_b[:], clusT[:, j:j + 1], None,
                            mybir.AluOpType.is_equal,
                        )
                        nc.tensor.matmul(pcnt[:], ones_col[:], oh[:],
                                         start=(j == 0), stop=(j == NJ - 1))
                        xall.append(xj)
                        ohall.append(oh)
                    # trickle k/v weights in (bf16 cast-DMA, one slab each/chunk)
                    nc.gpsimd.dma_start(out=wk_sb[ch][:],
                                        in_=wk_d[ch * P:(ch + 1) * P, :])
                    nc.gpsimd.dma_start(out=wv_sb[ch][:],
                                        in_=wv_d[ch * P:(ch + 1) * P, :])
                    # x^T for this chunk: PE transposes, evict-cast to fp8
                    xt8 = xtp8.tile([P, NK * TCH], F8, tag="xt8")
                    xt8v = xt8.rearrange("p (k t) -> p k t", t=TCH)
                    for k in range(NK):
                        for jj in range(NMT):
                            pt = psTT.tile([P, P], BF16, tag="ptt")
                            nc.tensor.transpose(
                                pt[:], xall[ch * NMT + jj][:, k * P:(k + 1) * P],
                                ident128[:],
                            )
                            nc.vector.tensor_copy(
                                xt8v[:, k:k + 1, jj * P:(jj + 1) * P], pt[:]
                            )
                    # qh^T for this chunk (fp8 DoubleRow over d)
                    for m in range(NK):
                        pq = psq.tile([P, TCH], FP32, tag="pq")
                        for j2 in range(NK // 2):
                            nc.tensor.matmul(
                                pq[:],
                                wq8v[:, 2 * j2:2 * j2 + 2, m * P:(m + 1) * P],
                                xt8v[:, 2 * j2:2 * j2 + 2, :],
                                start=(j2 == 0), stop=(j2 == NK // 2 - 1),
                                perf_mode=DR,
                            )
                        nc.scalar.copy(qh8[ch * NK + m][:], pq[:])

                # counts -> inv row (for k means) + raw column (for vc')
                nc.scalar.copy(cnt_sb[:], pcnt[:])
                nc.vector.tensor_copy(cnt_bf[:], cnt_sb[:])
                nc.vector.tensor_scalar_max(cm_row[:], cnt_sb[:], 0.5)
                nc.vector.reciprocal(inv_row[:], cm_row[:])
                nc.gpsimd.partition_broadcast(inv_bc[:], inv_row[:])
                for mc in range(2):
                    pt = psT.tile([P, 1], BF16, tag="pt")
                    nc.tensor.matmul(
                        pt[:], cnt_bf[0:1, mc * P:(mc + 1) * P], one11[:],
                        is_transpose=True,
                    )
                    nc.scalar.copy(cnt_col[:, mc:mc + 1], pt[:])

                # xc^T (raw cluster sums, d-major) and mean version for kc
                for m in range(NK):
                    pxc = psAm.tile([P, C], FP32, tag="pxc")
                    for j in range(NJ):
                        nc.tensor.matmul(
                            pxc[:], xall[j][:, m * P:(m + 1) * P], ohall[j][:],
                            start=(j == 0), stop=(j == NJ - 1),
                        )
                    nc.vector.tensor_mul(xcm[m][:], pxc[:], inv_bc[:])
                    nc.scalar.copy(xcr[m][:], pxc[:])

            # w_proj tiles live only from phase B on (SBUF headroom in A)
            with tc.tile_pool(name="wpp", bufs=1) as wpp:
                wp_sb = [wpp.tile([P, D], BF16, tag=f"wp{k}", name=f"wp{k}")
                         for k in range(NK)]
                for k in range(NK):
                    nc.gpsimd.dma_start(out=wp_sb[k][:],
                                        in_=wp_d[k * P:(k + 1) * P, :])

                # ---- phase B: kc^T, vc' = xc @ w_v (with cnt column) ----
                with (
                    tc.tile_pool(name="psBk", bufs=2, space="PSUM") as psBk,
                    tc.tile_pool(name="psBv", bufs=2, space="PSUM") as psBv,
                ):
                    for m in range(NK):
                        pk = psBk.tile([P, C], FP32, tag="pk")
                        for k in range(NK):
                            nc.tensor.matmul(
                                pk[:], wk_sb[k][:, m * P:(m + 1) * P], xcm[k][:],
                                start=(k == 0), stop=(k == NK - 1),
                            )
                        nc.vector.tensor_copy(kc8[m][:], pk[:])
                    for mc in range(2):
                        va = vca[mc].rearrange("p (h e) -> p h e", e=65)
                        nc.vector.tensor_scalar(
                            va[:, :, 64:65], ones16v[:], cnt_col[:, mc:mc + 1],
                            None, mybir.AluOpType.mult,
                        )
                        for nn in range(2):
                            pv = psBv.tile([P, 512], FP32, tag="pv")
                            for k in range(NK):
                                nc.tensor.matmul(
                                    pv[:], xcr[k][:, mc * P:(mc + 1) * P],
                                    wv_sb[k][:, nn * 512:(nn + 1) * 512],
                                    start=(k == 0), stop=(k == NK - 1),
                                )
                            nc.vector.tensor_copy(
                                va[:, nn * 8:(nn + 1) * 8, 0:64],
                                pv.rearrange("p (h e) -> p h e", e=64),
                            )

                # ---- phase C: software-pipelined attention + output proj ----
                with (
                    tc.tile_pool(name="exq", bufs=3) as exq,
                    tc.tile_pool(name="otp", bufs=2) as otp,
                    tc.tile_pool(name="sep", bufs=2) as sep,
                    tc.tile_pool(name="finp", bufs=3) as finp,
                    tc.tile_pool(name="pss", bufs=2, space="PSUM") as pss,
                    tc.tile_pool(name="psav", bufs=2, space="PSUM") as psav,
                    tc.tile_pool(name="psf", bufs=2, space="PSUM") as psf,
                ):
                    def emit_scores_pair(ch, hp, st):
                        # two heads, alternating PE row-groups (0-63 / 64-127)
                        # so consecutive matmuls overlap in the array
                        s2s = [pss.tile([P, 2 * TCH], FP32, tag="s", name=f"s{i}")
                               for i in range(2)]
                        for mc in range(2):
                            for par in range(2):
                                s3 = s2s[par].rearrange("p (m t) -> p m t", t=TCH)
                                nc.tensor.matmul(
                                    s3[:, mc:mc + 1, :],
                                    kc8[hp][par * 64:par * 64 + 64,
                                            mc * P:(mc + 1) * P],
                                    qh8[ch * NK + hp][par * 64:par * 64 + 64, :],
                                    start=True, stop=True,
                                )
                        for par in range(2):
                            ex2 = exq.tile([P, 2 * TCH], BF16, tag="ex")
                            nc.scalar.activation(
                                ex2[:], s2s[par][:],
                                mybir.ActivationFunctionType.Exp, scale=0.125,
                            )
                            st["ex"].append(ex2)

                    def emit_av_head(ch, h, st):
                        m, off = h // 2, (h % 2) * 64
                        ex3 = st["ex"][h].rearrange("p (m t) -> p m t", t=TCH)
                        pav = psav.tile([65, TCH], FP32, tag="av")
                        for mc in range(2):
                            nc.tensor.matmul(
                                pav[:], vca[mc][:, h * 65:(h + 1) * 65],
                                ex3[:, mc:mc + 1, :],
                                start=(mc == 0), stop=(mc == 1),
                            )
                        se_dst = st["se"][0:1, h * TCH:(h + 1) * TCH]
                        nc.vector.tensor_copy(se_dst, pav[64:65, :])
                        if h % 2 == 0:
                            nc.scalar.copy(st["outT"][m][off:off + 64, :],
                                           pav[0:64, :])
                        else:
                            nc.vector.tensor_copy(st["outT"][m][off:off + 64, :],
                                                  pav[0:64, :])

                    def emit_recip(st):
                        # 1/sumexp per (head, token) via one SBUF round trip
                        sq = sep.tile([P, H * TCH // P], FP32, tag="sq")
                        nc.gpsimd.dma_start(
                            out=sq[:],
                            in_=st["se"].rearrange("a (p t) -> a p t", t=TCH),
                        )
                        rq = sep.tile([P, H * TCH // P], FP32, tag="rq")
                        nc.vector.reciprocal(rq[:], sq[:])
                        rec = sep.tile([1, H * TCH], BF16, tag="rec")
                        nc.gpsimd.dma_start(
                            out=rec.rearrange("a (p t) -> a p t", t=TCH),
                            in_=rq[:],
                        )
                        st["rec"] = rec

                    def emit_pbc_muls(st):
                        rec = st["rec"]
                        for m in range(NK):
                            pbc = psf.tile([P, TCH], FP32, tag="pf")
                            for par in range(2):
                                h = 2 * m + par
                                nc.tensor.matmul(
                                    pbc[par * 64:(par + 1) * 64, :], ones_row[:],
                                    rec[0:1, h * TCH:(h + 1) * TCH],
                                    start=True, stop=True,
                                )
                            nc.vector.tensor_mul(st["outT"][m][:],
                                                 st["outT"][m][:], pbc[:])

                    def emit_proj_mt(st, mt):
                        t0 = st["ch"] * TCH
                        pf = [psf.tile([P, 512], FP32, tag="pf", name=f"pf{nn}")
                              for nn in range(2)]
                        for k in range(NK):
                            for nn in range(2):
                                nc.tensor.matmul(
                                    pf[nn][:],
                                    st["outT"][k][:, mt * P:(mt + 1) * P],
                                    wp_sb[k][:, nn * 512:(nn + 1) * 512],
                                    start=(k == 0), stop=(k == NK - 1),
                                )
                        for nn in range(2):
                            fin = finp.tile([P, 512], FP32, tag="fin")
                            nc.vector.tensor_add(
                                fin[:], pf[nn][:],
                                b_bc[:, nn * 512:(nn + 1) * 512]
                            )
                            nc.sync.dma_start(
                                out=out_d[t0 + mt * P:t0 + (mt + 1) * P,
                                          nn * 512:(nn + 1) * 512],
                                in_=fin[:],
                            )

                    prev = None
                    for ch in range(NCH + 1):
                        if ch < NCH:
                            st = {
                                "ch": ch,
                                "ex": [],
                                "outT": [otp.tile([P, TCH], BF16, tag=f"ot{m}",
                                                  name=f"ot{m}")
                                         for m in range(NK)],
                                "se": sep.tile([1, H * TCH], BF16, tag="se",
                                               name="se"),
                            }
                            for hp in range(H // 2):
                                # prev chunk's deferred work first (no deps on
                                # this chunk) to keep the PE stream dense
                                if prev is not None:
                                    if hp == 1:
                                        emit_pbc_muls(prev)
                                    elif hp in (2, 4, 5, 6):
                                        mt = {2: 0, 4: 1, 5: 2, 6: 3}[hp]
                                        emit_proj_mt(prev, mt)
                                emit_scores_pair(ch, hp, st)
                                if hp > 0:
                                    emit_av_head(ch, 2 * hp - 2, st)
                                    emit_av_head(ch, 2 * hp - 1, st)
                            emit_av_head(ch, H - 2, st)
                            emit_av_head(ch, H - 1, st)
                            emit_recip(st)
                            prev = st
                        else:
                            emit_pbc_muls(prev)
                            for mt in range(NMT):
                                emit_proj_mt(prev, mt)
    nc.compile()
    return nc


_NC = None


def _get_nc():
    global _NC
    if _NC is None:
        _NC = build_nc()
    return _NC


def make_in_maps(cluster, q, w_q, w_kv, w_proj, b_proj):
    cluster = np.ascontiguousarray(np.asarray(cluster).astype(np.int32, copy=False))
    q = np.asarray(q, dtype=np.float32)
    w_q = np.ascontiguousarray(np.asarray(w_q, dtype=np.float32))
    w_kv = np.asarray(w_kv, dtype=np.float32)
    w_k = np.ascontiguousarray(w_kv[:, :D])
    w_v = np.ascontiguousarray(w_kv[:, D:])
    w_proj = np.ascontiguousarray(np.asarray(w_proj, dtype=np.float32))
    b_proj = np.ascontiguousarray(
        np.asarray(b_proj, dtype=np.float32).reshape(1, D)
    )
    return [
        {
            "x": np.ascontiguousarray(q[i]),
            "cluster": cluster[i],
            "w_q": w_q,
            "w_k": w_k,
            "w_v": w_v,
            "w_proj": w_proj,
            "b_proj": b_proj,
        }
        for i in range(q.shape[0])
    ]


def kernel(cluster, q, w_q, w_kv, w_proj, b_proj):
    global LAST_RESULTS
    from concourse.bass_utils import run_bass_kernel_spmd

    nc = _get_nc()
    in_maps = make_in_maps(cluster, q, w_q, w_kv, w_proj, b_proj)
    ncores = len(in_maps)
    res = run_bass_kernel_spmd(
        nc, in_maps, core_ids=list(range(ncores)), trace=TRACE
    )
    LAST_RESULTS = res
    return np.stack([res.results[i]["out"] for i in range(ncores)], axis=0)

